# revision 2
# baseline (speedup 1.0000x reference)
import os
import sys

import numpy as np

for _p in ("/opt/trn_rl_repo",):
    if _p not in sys.path:
        sys.path.insert(0, _p)

import concourse.bass as bass
import concourse.bacc as bacc
import concourse.mybir as mybir
from concourse import tile
from concourse.bass_utils import run_bass_kernel_spmd

NH, HD, MB = 8, 32, 16
GROUPS = 32
B, C, H, W = 8, 256, 64, 64
HW = H * W
CIN = 512
F32 = mybir.dt.float32

_cache = {}
LAST = None  # last BassKernelResults (exec_time_ns when BASS_TRACE=1)


def _build_nc():
    """Per-core kernel: one batch sample.

    Computes, on device:
      o1  = relu((w_c1 @ xcat) * scale1 + bias1)   (conv1 1x1 + BN + ReLU)
      tmp = wr @ xcat + br                          (grouped reduce-in 1x1 conv)
    xcat: [512, 4096], weights pre-transposed host-side to lhsT layout [K, M].
    """
    nc = bacc.Bacc(None, target_bir_lowering=False, debug=False)
    xcat = nc.declare_dram_parameter("xcat", [CIN, HW], F32, isOutput=False)
    w1t = nc.declare_dram_parameter("w1t", [CIN, C], F32, isOutput=False)
    wrt = nc.declare_dram_parameter("wrt", [CIN, C], F32, isOutput=False)
    sbm = nc.declare_dram_parameter("sbm", [C, 3], F32, isOutput=False)
    o1 = nc.declare_dram_parameter("o1", [C, HW], F32, isOutput=True)
    tmp = nc.declare_dram_parameter("tmp", [C, HW], F32, isOutput=True)

    KT = CIN // 128   # 4 contraction tiles
    MT = C // 128     # 2 output-row tiles
    NFREE = 512
    NT = HW // NFREE  # 8 column chunks

    with tile.TileContext(nc) as tc:
        with (
            tc.tile_pool(name="xp", bufs=1) as xp,
            tc.tile_pool(name="wp", bufs=1) as wp,
            tc.tile_pool(name="cp", bufs=1) as cp,
            tc.tile_pool(name="op", bufs=3) as op,
            tc.tile_pool(name="ps", bufs=3, space=bass.MemorySpace.PSUM) as ps,
        ):
            xt = []
            for k in range(KT):
                t = xp.tile([128, HW], F32, tag=f"x{k}")
                nc.sync.dma_start(t[:], xcat[k * 128:(k + 1) * 128, :])
                xt.append(t)
            w1tl, wrtl = [], []
            for k in range(KT):
                t = wp.tile([128, C], F32, tag=f"w1_{k}")
                nc.sync.dma_start(t[:], w1t[k * 128:(k + 1) * 128, :])
                w1tl.append(t)
                t2 = wp.tile([128, C], F32, tag=f"wr_{k}")
                nc.sync.dma_start(t2[:], wrt[k * 128:(k + 1) * 128, :])
                wrtl.append(t2)
            sb = cp.tile([128, 3 * MT], F32)
            for m in range(MT):
                nc.sync.dma_start(sb[:, m * 3:(m + 1) * 3],
                                  sbm[m * 128:(m + 1) * 128, :])

            for n in range(NT):
                for m in range(MT):
                    acc = ps.tile([128, NFREE], F32, tag="acc")
                    for k in range(KT):
                        nc.tensor.matmul(
                            acc[:],
                            w1tl[k][:, m * 128:(m + 1) * 128],
                            xt[k][:, n * NFREE:(n + 1) * NFREE],
                            start=(k == 0), stop=(k == KT - 1))
                    ot = op.tile([128, NFREE], F32, tag="ot")
                    nc.scalar.activation(
                        ot[:], acc[:], mybir.ActivationFunctionType.Relu,
                        bias=sb[:, m * 3 + 1:m * 3 + 2],
                        scale=sb[:, m * 3 + 0:m * 3 + 1])
                    nc.sync.dma_start(
                        o1[m * 128:(m + 1) * 128, n * NFREE:(n + 1) * NFREE],
                        ot[:])

                    acc2 = ps.tile([128, NFREE], F32, tag="acc2")
                    for k in range(KT):
                        nc.tensor.matmul(
                            acc2[:],
                            wrtl[k][:, m * 128:(m + 1) * 128],
                            xt[k][:, n * NFREE:(n + 1) * NFREE],
                            start=(k == 0), stop=(k == KT - 1))
                    ot2 = op.tile([128, NFREE], F32, tag="ot2")
                    nc.vector.tensor_scalar_add(
                        ot2[:], acc2[:], sb[:, m * 3 + 2:m * 3 + 3])
                    nc.sync.dma_start(
                        tmp[m * 128:(m + 1) * 128, n * NFREE:(n + 1) * NFREE],
                        ot2[:])
    nc.compile()
    return nc


def _group_dense(w):
    """[256,16,1,1] grouped (groups=32) 1x1 conv weight -> dense [256,512]."""
    wd = np.zeros((C, CIN), np.float32)
    opg, ipg = C // GROUPS, CIN // GROUPS
    for g in range(GROUPS):
        wd[g * opg:(g + 1) * opg, g * ipg:(g + 1) * ipg] = w[g * opg:(g + 1) * opg, :, 0, 0]
    return wd


def _bn_scale(g):
    return (np.asarray(g, np.float64) / np.sqrt(1.0 + 1e-5)).astype(np.float32)


def _bn_relu(x, g, b):
    s = _bn_scale(g)
    return np.maximum(x * s[None, :, None, None] + b[None, :, None, None], 0.0)


def _conv_shift(x, w, pads, dil=1):
    """NCHW conv via shifted channel-matmuls. w: [O,I,kh,kw], stride 1."""
    O, I, kh, kw = w.shape
    ph, pw = pads
    xp = np.pad(x, ((0, 0), (0, 0), (ph, ph), (pw, pw)))
    out = np.zeros((x.shape[0], O, H, W), np.float32)
    for a in range(kh):
        for b_ in range(kw):
            sl = xp[:, :, a * dil:a * dil + H, b_ * dil:b_ * dil + W]
            sl = np.ascontiguousarray(sl).reshape(x.shape[0], I, HW)
            out += np.matmul(w[:, :, a, b_][None], sl).reshape(x.shape[0], O, H, W)
    return out


def _layer_norm(x, g, b, eps):
    mu = x.mean(-1, keepdims=True, dtype=np.float32)
    var = ((x - mu) ** 2).mean(-1, keepdims=True, dtype=np.float32)
    return (x - mu) / np.sqrt(var + eps) * g + b


def _ln_fused_l2_bwd(x, target, g, b, eps=1e-6):
    D = x.shape[-1]
    mu = x.mean(-1, keepdims=True, dtype=np.float32)
    var = ((x - mu) ** 2).mean(-1, keepdims=True, dtype=np.float32)
    std = np.sqrt(var + eps)
    xh = (x - mu) / std
    gy = (g * xh + b - target) * g
    return (D * gy - gy.sum(-1, keepdims=True)
            - xh * (gy * xh).sum(-1, keepdims=True)) / (D * std)


def _rotate_half(x):
    x1, x2 = np.split(x, 2, -1)
    return np.concatenate([-x2, x1], -1)


def _ttt_linear(hs, wq, wk, wv, wo, W1p, b1p, ln_w, ln_b, lr_w, lr_b, tok_idx,
                pg, pb):
    Bb, L, D = hs.shape
    nm = L // MB
    heads = lambda x: x.reshape(Bb, L, NH, HD).transpose(0, 2, 1, 3)
    XQ, XK, XV = heads(hs @ wq.T), heads(hs @ wk.T), heads(hs @ wv.T)
    inv = (1.0 / (10000.0 ** (np.arange(0, HD, 2, dtype=np.float32) / HD))).astype(np.float32)
    f = np.arange(L, dtype=np.float32)[:, None] * inv[None, :]
    emb = np.concatenate([f, f], -1)
    cos, sin = np.cos(emb), np.sin(emb)
    XQ = XQ * cos + _rotate_half(XQ) * sin
    XK = XK * cos + _rotate_half(XK) * sin
    mb5 = lambda x: x.reshape(Bb, NH, nm, MB, HD)
    XQ, XK, XV = mb5(XQ), mb5(XK), mb5(XV)
    Xmb = hs.reshape(Bb, nm, MB, D)
    ttt_lr = 1.0 / (1.0 + np.exp(-(np.einsum("bnkc,hoc->bhnko", Xmb, lr_w)
                                   + lr_b.reshape(1, NH, 1, 1, 1))))
    ttt_lr_eta = (1.0 / HD) * ttt_lr.transpose(0, 1, 2, 4, 3)
    token = np.clip(1.0 / np.arange(1, MB + 1, dtype=np.float32) + tok_idx, 0.0, None)
    eta = token.reshape(1, 1, 1, MB, 1) * ttt_lr_eta          # [B,NH,nm,MB,MB]
    gamma = ln_w.reshape(1, NH, 1, HD)
    beta = ln_b.reshape(1, NH, 1, HD)
    tril = np.tril(np.ones((MB, MB), np.float32))

    Wc = np.broadcast_to(W1p[None], (Bb, NH, HD, HD)).astype(np.float32).copy()
    bc = np.broadcast_to(b1p[None], (Bb, NH, 1, HD)).astype(np.float32).copy()
    outs = np.empty((Bb, NH, nm, MB, HD), np.float32)
    for t in range(nm):
        xq, xk, xv = XQ[:, :, t], XK[:, :, t], XV[:, :, t]
        et = eta[:, :, t]
        Z1 = xk @ Wc + bc
        grad = _ln_fused_l2_bwd(Z1, xv - xk, gamma, beta)
        Attn = tril * (xq @ xk.transpose(0, 1, 3, 2))
        b_bar = bc - (tril * et) @ grad
        Z1_bar = xq @ Wc - (et * Attn) @ grad + b_bar
        last = et[:, :, -1, :, None]
        Wc = Wc - (last * xk).transpose(0, 1, 3, 2) @ grad
        bc = bc - np.sum(last * grad, -2, keepdims=True)
        outs[:, :, t] = xq + _layer_norm(Z1_bar, gamma, beta, 1e-6)
    out = outs.transpose(0, 2, 3, 1, 4).reshape(Bb, L, D)
    out = _layer_norm(out, pg, pb, 1e-6)
    return out @ wo.T


def kernel(**inputs):
    f = {k: (np.asarray(v, np.float32) if np.asarray(v).dtype.kind == "f"
             else np.asarray(v)) for k, v in inputs.items()}

    dense_cat = np.concatenate(
        [f["dense_embeddings_boundary"], f["dense_embeddings_box"]], 1)
    xflat = dense_cat.reshape(B, CIN, HW)

    w1 = np.ascontiguousarray(f["w_c1"][:, :, 0, 0])        # [256,512]
    wr = _group_dense(f["w_red_in"])
    scale1 = _bn_scale(f["g_bn1"])
    sbm = np.stack([scale1, f["be_bn1"], f["b_red_in"]], 1).astype(np.float32)
    w1t = np.ascontiguousarray(w1.T)
    wrt = np.ascontiguousarray(wr.T)

    if "nc" not in _cache:
        _cache["nc"] = _build_nc()
    nc = _cache["nc"]

    in_maps = [{"xcat": np.ascontiguousarray(xflat[b]), "w1t": w1t,
                "wrt": wrt, "sbm": sbm} for b in range(B)]
    global LAST
    LAST = run_bass_kernel_spmd(nc, in_maps, list(range(8)))
    res = LAST.results
    x = np.stack([res[b]["o1"] for b in range(B)]).reshape(B, C, H, W)
    tmp = np.stack([res[b]["tmp"] for b in range(B)]).reshape(B, C, H, W)

    x = _bn_relu(_conv_shift(x, f["w_c2"], (0, 1)), f["g_bn2"], f["be_bn2"])
    x = _bn_relu(_conv_shift(x, f["w_c3"], (1, 0)), f["g_bn3"], f["be_bn3"])
    x = _bn_relu(_conv_shift(x, f["w_c4"], (3, 3), dil=3), f["g_bn4"], f["be_bn4"])
    dense_em = x + tmp

    hf = f["high_frequency"]
    hs = hf.reshape(B, C, HW).transpose(0, 2, 1)
    hs = _layer_norm(hs, f["ln_g"], f["ln_b"], 1e-5)
    hf2 = _ttt_linear(hs, f["wq"], f["wk"], f["wv"], f["wo"], f["W1"], f["b1"],
                      f["ttt_ln_w"], f["ttt_ln_b"], f["lr_w"], f["lr_b"],
                      f["tok_idx"], f["post_ln_g"], f["post_ln_b"])
    hf2 = hf2.transpose(0, 2, 1).reshape(B, C, H, W)

    de = np.concatenate([dense_em, hf2], 1)
    wr2 = _group_dense(f["w_red_out2"])
    de = (np.matmul(wr2[None], de.reshape(B, CIN, HW)).reshape(B, C, H, W)
          + f["b_red_out2"][None, :, None, None])
    return de, f["sparse_embeddings_box"]


# revision 5
# speedup vs baseline: 1.4136x; 1.4136x over previous
import os
import sys

import numpy as np

for _p in ("/opt/trn_rl_repo",):
    if _p not in sys.path:
        sys.path.insert(0, _p)

import concourse.bass as bass
import concourse.bacc as bacc
import concourse.mybir as mybir
from concourse import tile
from concourse.bass_utils import run_bass_kernel_spmd

NH, HD, MB = 8, 32, 16
GROUPS = 32
B, C, H, W = 8, 256, 64, 64
HW = H * W
CIN = 512
F32 = mybir.dt.float32

_cache = {}
LAST = None  # last BassKernelResults (exec_time_ns when BASS_TRACE=1)


def _build_nc():
    """Per-core kernel: one batch sample, full conv stack on device.

    dense_em = relu(bn4(conv4_3x3d3(relu(bn3(conv3_3x1(relu(bn2(conv2_1x3(
               relu(bn1(conv1_1x1(xcat))))))))))) + (wr @ xcat + br)
    xcat: [512, 4096]; weights pre-transposed host-side to lhsT [K, M].
    Spatial convs are shifted channel-matmuls with 2D (row, col) APs over
    the flat [C, 64*64] layout; 8-row (512-element) PSUM chunks.
    """
    nc = bacc.Bacc(None, target_bir_lowering=False, debug=False)
    xcat = nc.declare_dram_parameter("xcat", [CIN, HW], F32, isOutput=False)
    w1t = nc.declare_dram_parameter("w1t", [CIN, C], F32, isOutput=False)
    wrt = nc.declare_dram_parameter("wrt", [CIN, C], F32, isOutput=False)
    w2t = nc.declare_dram_parameter("w2t", [3, C, C], F32, isOutput=False)
    w3t = nc.declare_dram_parameter("w3t", [3, C, C], F32, isOutput=False)
    w4t = nc.declare_dram_parameter("w4t", [9, C, C], F32, isOutput=False)
    sbm = nc.declare_dram_parameter("sbm", [C, 9], F32, isOutput=False)
    dem = nc.declare_dram_parameter("dense_em", [C, HW], F32, isOutput=True)

    KT = CIN // 128   # 4 contraction tiles for conv1
    KT2 = C // 128    # 2 contraction tiles for conv2-4
    MT = C // 128     # 2 output-row tiles
    NFREE = 512
    NT = HW // NFREE  # 8 chunks of 8 image rows
    NSB = 9

    with tile.TileContext(nc) as tc:
        with (
            tc.tile_pool(name="xp", bufs=1) as xp,
            tc.tile_pool(name="ip", bufs=1) as ip,
            tc.tile_pool(name="wp", bufs=1) as wp,
            tc.tile_pool(name="cp", bufs=1) as cp,
            tc.tile_pool(name="op", bufs=3) as op,
            tc.tile_pool(name="ps1", bufs=1, space=bass.MemorySpace.PSUM) as ps1,
            tc.tile_pool(name="ps2", bufs=2, space=bass.MemorySpace.PSUM) as ps2,
        ):
            xt = []
            for k in range(KT):
                t = xp.tile([128, HW], F32, tag=f"x{k}")
                nc.sync.dma_start(t[:], xcat[k * 128:(k + 1) * 128, :])
                xt.append(t)
            w1tl, wrtl = [], []
            for k in range(KT):
                t = wp.tile([128, C], F32, tag=f"w1_{k}")
                nc.sync.dma_start(t[:], w1t[k * 128:(k + 1) * 128, :])
                w1tl.append(t)
                t2 = wp.tile([128, C], F32, tag=f"wr_{k}")
                nc.sync.dma_start(t2[:], wrt[k * 128:(k + 1) * 128, :])
                wrtl.append(t2)

            def load_w(param, ntaps, label):
                tiles = []
                for i in range(ntaps):
                    row = []
                    for k in range(KT2):
                        t = wp.tile([128, C], F32, tag=f"{label}_{i}_{k}")
                        nc.sync.dma_start(
                            t[:], param[i, k * 128:(k + 1) * 128, :])
                        row.append(t)
                    tiles.append(row)
                return tiles

            w2l = load_w(w2t, 3, "w2")
            w3l = load_w(w3t, 3, "w3")
            w4l = load_w(w4t, 9, "w4")

            sb = cp.tile([128, NSB * MT], F32)
            for m in range(MT):
                nc.sync.dma_start(sb[:, m * NSB:(m + 1) * NSB],
                                  sbm[m * 128:(m + 1) * 128, :])

            # persistent stage outputs [128, 4096] x2 (m tiles)
            x1 = [ip.tile([128, HW], F32, tag=f"s1_{m}", name=f"x1_{m}")
                  for m in range(MT)]
            tmp = [ip.tile([128, HW], F32, tag=f"tmp_{m}", name=f"tmp_{m}")
                   for m in range(MT)]
            # x2 / x3 reuse xcat slots (conv1 is done reading them by then)
            x2 = [xp.tile([128, HW], F32, tag=f"x{m}", name=f"x2_{m}")
                  for m in range(MT)]
            x3 = [xp.tile([128, HW], F32, tag=f"x{2 + m}", name=f"x3_{m}")
                  for m in range(MT)]

            # ---- conv1 (1x1, K=512) + tmp (grouped reduce as dense matmul)
            for n in range(NT):
                sl = slice(n * NFREE, (n + 1) * NFREE)
                for m in range(MT):
                    acc = ps1.tile([128, NFREE], F32, tag="acc")
                    for k in range(KT):
                        nc.tensor.matmul(
                            acc[:], w1tl[k][:, m * 128:(m + 1) * 128],
                            xt[k][:, sl], start=(k == 0), stop=(k == KT - 1))
                    nc.scalar.activation(
                        x1[m][:, sl], acc[:], mybir.ActivationFunctionType.Relu,
                        bias=sb[:, m * NSB + 1:m * NSB + 2],
                        scale=sb[:, m * NSB + 0:m * NSB + 1])

                    acc2 = ps1.tile([128, NFREE], F32, tag="acc2")
                    for k in range(KT):
                        nc.tensor.matmul(
                            acc2[:], wrtl[k][:, m * 128:(m + 1) * 128],
                            xt[k][:, sl], start=(k == 0), stop=(k == KT - 1))
                    nc.vector.tensor_scalar_add(
                        tmp[m][:, sl], acc2[:],
                        sb[:, m * NSB + 2:m * NSB + 3])

            # ---- spatial conv stage helper ----------------------------
            def conv_stage(src, wtl, taps, dst, sb_off, acc_tag, add_tmp):
                srcv = [s[:].rearrange("p (h w) -> p h w", w=W) for s in src]
                for n in range(NT):
                    h0 = n * 8
                    for m in range(MT):
                        acc = ps2.tile([128, 8, W], F32, tag=acc_tag)
                        first = True
                        for ti, (dh, dw) in enumerate(taps):
                            lo = max(h0, -dh)
                            hi = min(h0 + 8, H - dh)
                            c0, c1 = max(0, -dw), min(W, W - dw)
                            if lo >= hi:
                                continue
                            last_tap = ti == len(taps) - 1
                            for k in range(KT2):
                                nc.tensor.matmul(
                                    acc[:, lo - h0:hi - h0, c0:c1],
                                    wtl[ti][k][:, m * 128:(m + 1) * 128],
                                    srcv[k][:, lo + dh:hi + dh,
                                            c0 + dw:c1 + dw],
                                    start=first,
                                    stop=last_tap and k == KT2 - 1,
                                    skip_group_check=True)
                                first = False
                        sl = slice(n * NFREE, (n + 1) * NFREE)
                        flat = acc[:].rearrange("p h w -> p (h w)")
                        if add_tmp is None:
                            nc.scalar.activation(
                                dst[m][:, sl], flat,
                                mybir.ActivationFunctionType.Relu,
                                bias=sb[:, m * NSB + sb_off + 1:
                                        m * NSB + sb_off + 2],
                                scale=sb[:, m * NSB + sb_off:
                                         m * NSB + sb_off + 1])
                        else:
                            rt = op.tile([128, NFREE], F32, tag="rt")
                            nc.scalar.activation(
                                rt[:], flat,
                                mybir.ActivationFunctionType.Relu,
                                bias=sb[:, m * NSB + sb_off + 1:
                                        m * NSB + sb_off + 2],
                                scale=sb[:, m * NSB + sb_off:
                                         m * NSB + sb_off + 1])
                            ot = op.tile([128, NFREE], F32, tag="ot")
                            nc.vector.tensor_add(
                                ot[:], rt[:], add_tmp[m][:, sl])
                            nc.sync.dma_start(
                                dem[m * 128:(m + 1) * 128, sl], ot[:])

            # taps ordered with the full-coverage center tap FIRST so the
            # start=True matmul initializes every PSUM element.
            t2 = [(0, 0), (0, -1), (0, 1)]
            t3 = [(0, 0), (-1, 0), (1, 0)]
            t4 = [(0, 0)] + [(dh, dw) for dh in (-3, 0, 3) for dw in (-3, 0, 3)
                             if not (dh == 0 and dw == 0)]
            # weight tap index must match tap order: build index maps
            w2o = [w2l[1], w2l[0], w2l[2]]            # dw=0,-1,+1 -> kw=1,0,2
            w3o = [w3l[1], w3l[0], w3l[2]]            # dh=0,-1,+1 -> kh=1,0,2
            w4o = [w4l[(dh // 3 + 1) * 3 + (dw // 3 + 1)] for (dh, dw) in t4]

            conv_stage(x1, w2o, t2, x2, 3, "c2", None)
            conv_stage(x2, w3o, t3, x3, 5, "c3", None)
            conv_stage(x3, w4o, t4, None, 7, "c4", tmp)
    nc.compile()
    return nc


def _group_dense(w):
    """[256,16,1,1] grouped (groups=32) 1x1 conv weight -> dense [256,512]."""
    wd = np.zeros((C, CIN), np.float32)
    opg, ipg = C // GROUPS, CIN // GROUPS
    for g in range(GROUPS):
        wd[g * opg:(g + 1) * opg, g * ipg:(g + 1) * ipg] = w[g * opg:(g + 1) * opg, :, 0, 0]
    return wd


def _bn_scale(g):
    return (np.asarray(g, np.float64) / np.sqrt(1.0 + 1e-5)).astype(np.float32)


def _bn_relu(x, g, b):
    s = _bn_scale(g)
    return np.maximum(x * s[None, :, None, None] + b[None, :, None, None], 0.0)


def _conv_shift(x, w, pads, dil=1):
    """NCHW conv via shifted channel-matmuls. w: [O,I,kh,kw], stride 1."""
    O, I, kh, kw = w.shape
    ph, pw = pads
    xp = np.pad(x, ((0, 0), (0, 0), (ph, ph), (pw, pw)))
    out = np.zeros((x.shape[0], O, H, W), np.float32)
    for a in range(kh):
        for b_ in range(kw):
            sl = xp[:, :, a * dil:a * dil + H, b_ * dil:b_ * dil + W]
            sl = np.ascontiguousarray(sl).reshape(x.shape[0], I, HW)
            out += np.matmul(w[:, :, a, b_][None], sl).reshape(x.shape[0], O, H, W)
    return out


def _layer_norm(x, g, b, eps):
    mu = x.mean(-1, keepdims=True, dtype=np.float32)
    var = ((x - mu) ** 2).mean(-1, keepdims=True, dtype=np.float32)
    return (x - mu) / np.sqrt(var + eps) * g + b


def _ln_fused_l2_bwd(x, target, g, b, eps=1e-6):
    D = x.shape[-1]
    mu = x.mean(-1, keepdims=True, dtype=np.float32)
    var = ((x - mu) ** 2).mean(-1, keepdims=True, dtype=np.float32)
    std = np.sqrt(var + eps)
    xh = (x - mu) / std
    gy = (g * xh + b - target) * g
    return (D * gy - gy.sum(-1, keepdims=True)
            - xh * (gy * xh).sum(-1, keepdims=True)) / (D * std)


def _rotate_half(x):
    x1, x2 = np.split(x, 2, -1)
    return np.concatenate([-x2, x1], -1)


def _ttt_linear(hs, wq, wk, wv, wo, W1p, b1p, ln_w, ln_b, lr_w, lr_b, tok_idx,
                pg, pb):
    Bb, L, D = hs.shape
    nm = L // MB
    heads = lambda x: x.reshape(Bb, L, NH, HD).transpose(0, 2, 1, 3)
    XQ, XK, XV = heads(hs @ wq.T), heads(hs @ wk.T), heads(hs @ wv.T)
    inv = (1.0 / (10000.0 ** (np.arange(0, HD, 2, dtype=np.float32) / HD))).astype(np.float32)
    f = np.arange(L, dtype=np.float32)[:, None] * inv[None, :]
    emb = np.concatenate([f, f], -1)
    cos, sin = np.cos(emb), np.sin(emb)
    XQ = XQ * cos + _rotate_half(XQ) * sin
    XK = XK * cos + _rotate_half(XK) * sin
    mb5 = lambda x: x.reshape(Bb, NH, nm, MB, HD)
    XQ, XK, XV = mb5(XQ), mb5(XK), mb5(XV)
    Xmb = hs.reshape(Bb, nm, MB, D)
    ttt_lr = 1.0 / (1.0 + np.exp(-(np.einsum("bnkc,hoc->bhnko", Xmb, lr_w)
                                   + lr_b.reshape(1, NH, 1, 1, 1))))
    ttt_lr_eta = (1.0 / HD) * ttt_lr.transpose(0, 1, 2, 4, 3)
    token = np.clip(1.0 / np.arange(1, MB + 1, dtype=np.float32) + tok_idx, 0.0, None)
    eta = token.reshape(1, 1, 1, MB, 1) * ttt_lr_eta          # [B,NH,nm,MB,MB]
    gamma = ln_w.reshape(1, NH, 1, HD)
    beta = ln_b.reshape(1, NH, 1, HD)
    tril = np.tril(np.ones((MB, MB), np.float32))

    Wc = np.broadcast_to(W1p[None], (Bb, NH, HD, HD)).astype(np.float32).copy()
    bc = np.broadcast_to(b1p[None], (Bb, NH, 1, HD)).astype(np.float32).copy()
    outs = np.empty((Bb, NH, nm, MB, HD), np.float32)
    for t in range(nm):
        xq, xk, xv = XQ[:, :, t], XK[:, :, t], XV[:, :, t]
        et = eta[:, :, t]
        Z1 = xk @ Wc + bc
        grad = _ln_fused_l2_bwd(Z1, xv - xk, gamma, beta)
        Attn = tril * (xq @ xk.transpose(0, 1, 3, 2))
        b_bar = bc - (tril * et) @ grad
        Z1_bar = xq @ Wc - (et * Attn) @ grad + b_bar
        last = et[:, :, -1, :, None]
        Wc = Wc - (last * xk).transpose(0, 1, 3, 2) @ grad
        bc = bc - np.sum(last * grad, -2, keepdims=True)
        outs[:, :, t] = xq + _layer_norm(Z1_bar, gamma, beta, 1e-6)
    out = outs.transpose(0, 2, 3, 1, 4).reshape(Bb, L, D)
    out = _layer_norm(out, pg, pb, 1e-6)
    return out @ wo.T


def kernel(**inputs):
    f = {k: (np.asarray(v, np.float32) if np.asarray(v).dtype.kind == "f"
             else np.asarray(v)) for k, v in inputs.items()}

    dense_cat = np.concatenate(
        [f["dense_embeddings_boundary"], f["dense_embeddings_box"]], 1)
    xflat = dense_cat.reshape(B, CIN, HW)

    w1 = np.ascontiguousarray(f["w_c1"][:, :, 0, 0])        # [256,512]
    wr = _group_dense(f["w_red_in"])
    sbm = np.stack([
        _bn_scale(f["g_bn1"]), f["be_bn1"], f["b_red_in"],
        _bn_scale(f["g_bn2"]), f["be_bn2"],
        _bn_scale(f["g_bn3"]), f["be_bn3"],
        _bn_scale(f["g_bn4"]), f["be_bn4"],
    ], 1).astype(np.float32)
    w1t = np.ascontiguousarray(w1.T)
    wrt = np.ascontiguousarray(wr.T)
    # [taps, K, M] lhsT layouts: w_t[tap, k, m] = w[m, k, kh, kw]
    w2t = np.ascontiguousarray(f["w_c2"][:, :, 0, :].transpose(2, 1, 0))
    w3t = np.ascontiguousarray(f["w_c3"][:, :, :, 0].transpose(2, 1, 0))
    w4t = np.ascontiguousarray(
        f["w_c4"].reshape(C, C, 9).transpose(2, 1, 0))

    if "nc" not in _cache:
        _cache["nc"] = _build_nc()
    nc = _cache["nc"]

    in_maps = [{"xcat": np.ascontiguousarray(xflat[b]), "w1t": w1t,
                "wrt": wrt, "w2t": w2t, "w3t": w3t, "w4t": w4t, "sbm": sbm}
               for b in range(B)]
    global LAST
    LAST = run_bass_kernel_spmd(nc, in_maps, list(range(8)))
    res = LAST.results
    dense_em = np.stack([res[b]["dense_em"] for b in range(B)]).reshape(
        B, C, H, W)

    hf = f["high_frequency"]
    hs = hf.reshape(B, C, HW).transpose(0, 2, 1)
    hs = _layer_norm(hs, f["ln_g"], f["ln_b"], 1e-5)
    hf2 = _ttt_linear(hs, f["wq"], f["wk"], f["wv"], f["wo"], f["W1"], f["b1"],
                      f["ttt_ln_w"], f["ttt_ln_b"], f["lr_w"], f["lr_b"],
                      f["tok_idx"], f["post_ln_g"], f["post_ln_b"])
    hf2 = hf2.transpose(0, 2, 1).reshape(B, C, H, W)

    de = np.concatenate([dense_em, hf2], 1)
    wr2 = _group_dense(f["w_red_out2"])
    de = (np.matmul(wr2[None], de.reshape(B, CIN, HW)).reshape(B, C, H, W)
          + f["b_red_out2"][None, :, None, None])
    return de, f["sparse_embeddings_box"]
